# revision 7
# baseline (speedup 1.0000x reference)
"""Trainium2 Bass kernel: batched causal single-head self-attention.

Reference computation (per batch b):
    q = x @ Wq; k = x @ Wk; v = x @ Wv          # [T, H] each, contraction over E
    S = (q @ k^T) / sqrt(H)                     # [T, T]
    P = softmax(causal_mask(S), axis=-1)
    out = P @ v                                 # [T, H]

Shapes: x [512, 256, 384] f32, W* [384, 64] f32, out [512, 256, 64] f32.
Sharding: pure data parallel, 64 batches per NeuronCore across 8 cores.

Device algorithm per batch (matmul operands bf16, fp32 PSUM accumulation):
  - host ships x^T quad-contiguous ([qd, p, s, c, t]) so each input DMA is
    128 descriptors x 6KB contiguous.
  - [k^T; q^T] = [Wk|Wq]^T @ xT  (one packed 128-wide stationary, 3 E-chunks,
    512-col moving = 2 batches per matmul). k^T lands on partitions 0:64 so
    the scores matmuls can use it directly as a K=64 stationary (no shift).
  - v^T  = Wv^T @ xT, then PE-transposed (ones row appended so the transposed
    tile carries a ones column -> softmax denominators for free).
  - S^T  = k^T-chunk.T @ q^T     ([tk, tq] layout; lower-left T/4 block skipped)
  - P    = exp(0.125 * S^T)      (ScalarE; no max-subtraction needed, |s|<~45)
  - P   *= causal 0/1 mask       (only the two diagonal 128x128 blocks)
  - out_aug[tq, 0:65] = sum_tk P[tk,tq] * [v|1][tk]; col 64 = denominator;
    divide + store. Output DMA'd in device layout [p, qd, s, j, h],
    un-scrambled on host.
  - 16 dummy 512-col matmuls at kernel start trip the PE HAM clock gate to
    2.4 GHz while the first input DMAs are still landing.
"""

import numpy as np
import ml_dtypes

B, T, E, H = 512, 256, 384, 64
NCORES = 8
BPC = B // NCORES  # 64
P = 128
EC = E // P  # 3
HP1 = H + 1  # 65

_cache: dict = {}


def _install_ntff_hook():
    """Shim antenv.axon_hooks (absent in this image) so run_bass_kernel_spmd
    trace=True can capture NTFF profiles via the axon .so's C ABI."""
    import contextlib
    import ctypes
    import sys
    import types

    if "antenv.axon_hooks" in sys.modules:
        return
    so_path = "/opt/axon/libaxon_pjrt.so"
    lib = ctypes.CDLL(so_path)
    if not hasattr(lib, "axon_start_nrt_profile"):
        return
    lib.axon_start_nrt_profile.argtypes = [
        ctypes.POINTER(ctypes.c_int64),
        ctypes.c_size_t,
    ]
    lib.axon_start_nrt_profile.restype = ctypes.c_int64
    lib.axon_stop_nrt_profile.argtypes = [ctypes.c_char_p]
    lib.axon_stop_nrt_profile.restype = ctypes.c_int64

    @contextlib.contextmanager
    def _hook(output_dir, device_ids):
        import jax

        jax.devices()
        if device_ids:
            ids = (ctypes.c_int64 * len(device_ids))(*device_ids)
            rc = lib.axon_start_nrt_profile(ids, len(device_ids))
        else:
            rc = lib.axon_start_nrt_profile(None, 0)
        if rc != 0:
            raise RuntimeError(f"axon_start_nrt_profile rc={rc}")
        try:
            yield
        finally:
            n = lib.axon_stop_nrt_profile(str(output_dir).encode())
            if n < 0:
                raise RuntimeError(f"axon_stop_nrt_profile rc={n}")
            print(f"profile: {n} file(s) written to {output_dir}", file=sys.stderr)

    mod = types.ModuleType("antenv.axon_hooks")
    _state = {"hook": _hook}
    mod.get_axon_ntff_profile_hook = lambda: _state["hook"]
    mod.set_axon_ntff_profile_hook = lambda h: _state.__setitem__("hook", h)
    sys.modules["antenv.axon_hooks"] = mod


def _build_program(bpc):
    import concourse.bacc as bacc
    import concourse.mybir as mybir
    import concourse.tile as tile

    f32 = mybir.dt.float32
    bf16 = mybir.dt.bfloat16
    Exp = mybir.ActivationFunctionType.Exp
    Mult = mybir.AluOpType.mult

    nc = bacc.Bacc(
        "TRN2",
        target_bir_lowering=False,
        debug=False,
        enable_asserts=False,
        num_devices=NCORES,
    )
    Q = 4
    assert bpc % Q == 0
    nquads = bpc // Q

    # quad-contiguous input: per partition 6KB contiguous per quad
    xt_d = nc.dram_tensor("xt", [nquads, P, Q, EC, T], bf16, kind="ExternalInput").ap()
    wkq_d = nc.dram_tensor("wkq", [P, EC, P], bf16, kind="ExternalInput").ap()
    wv_d = nc.dram_tensor("wv", [P, EC, H], bf16, kind="ExternalInput").ap()
    # 0/1 causal mask for the diagonal blocks of P^T tiles (tk<=tq keep)
    um_d = nc.dram_tensor("um", [P, P], bf16, kind="ExternalInput").ap()
    iden_d = nc.dram_tensor("iden", [HP1, HP1], bf16, kind="ExternalInput").ap()
    # device-layout output, un-scrambled host-side
    out_d = nc.dram_tensor("out", [P, nquads, Q, 2, H], f32, kind="ExternalOutput").ap()

    with tile.TileContext(nc) as tc:
        with (
            tc.tile_pool(name="const", bufs=1) as constp,
            tc.tile_pool(name="xin", bufs=4) as xpool,
            tc.tile_pool(name="qksb", bufs=3) as qkpool,
            tc.tile_pool(name="psb", bufs=3) as ppool,
            tc.tile_pool(name="vaug", bufs=3) as vpool,
            tc.tile_pool(name="osb", bufs=2) as opool,
            tc.tile_pool(name="rec", bufs=2) as rpool,
            tc.tile_pool(name="ps_qk", bufs=2, space="PSUM") as ps_qk,
            tc.tile_pool(name="ps_vt", bufs=2, space="PSUM") as ps_vt,
            tc.tile_pool(name="ps_s", bufs=2, space="PSUM") as ps_s,
            tc.tile_pool(name="ps_tr", bufs=1, space="PSUM") as ps_tr,
            tc.tile_pool(name="ps_o", bufs=1, space="PSUM") as ps_o,
        ):
            wkq = constp.tile([P, EC, P], bf16)
            nc.sync.dma_start(wkq, wkq_d)
            wv = constp.tile([P, EC, H], bf16)
            nc.sync.dma_start(wv, wv_d)
            um = constp.tile([P, P], bf16)
            nc.sync.dma_start(um, um_d)
            iden = constp.tile([HP1, HP1], bf16)
            nc.sync.dma_start(iden, iden_d)
            # v^T staging with a persistent ones row at partition 64 (manual
            # double-buffer so the ones row survives across iterations)
            vtabs = []
            for i in range(2):
                vt = constp.tile([HP1, 2, T], bf16, name=f"vtab{i}")
                nc.vector.memset(vt[H : H + 1, :, :], 1.0)
                vtabs.append(vt)

            # HAM warmup: ~3.4us of dummy matmuls (no DMA dependency) so the
            # PE clock gate is at 8/8 by the time real work arrives.
            wrm = constp.tile([P, 2, T], bf16, name="wrm")
            nc.vector.memset(wrm, 0.0)
            wp = ps_qk.tile([P, 2, T], f32, name="qk_ps")
            for i in range(16):
                nc.tensor.matmul(wp, wrm[:, 0, 0:P], wrm, start=True, stop=True)

            for qd in range(nquads):
                xt = xpool.tile([P, Q, EC, T], bf16)
                nc.sync.dma_start(xt, xt_d[qd])
                o_sb = opool.tile([P, Q, 2, H], f32)

                for prl in range(Q // 2):
                    s0 = 2 * prl
                    pr = qd * (Q // 2) + prl

                    qk_ps = ps_qk.tile([P, 2, T], f32)
                    vt_ps = ps_vt.tile([H, 2, T], f32)
                    for c in range(EC):
                        nc.tensor.matmul(
                            qk_ps,
                            wkq[:, c, :],
                            xt[:, s0 : s0 + 2, c, :],
                            start=(c == 0),
                            stop=(c == EC - 1),
                        )
                    for c in range(EC):
                        nc.tensor.matmul(
                            vt_ps,
                            wv[:, c, :],
                            xt[:, s0 : s0 + 2, c, :],
                            start=(c == 0),
                            stop=(c == EC - 1),
                        )

                    # k^T / q^T PSUM -> SBUF bf16 (base-partition-0 tiles so the
                    # scores matmul gets matching K=64 operands), split engines
                    k_sb = qkpool.tile([H, 2, T], bf16, name="k_sb")
                    q_sb = qkpool.tile([H, 2, T], bf16, name="q_sb")
                    nc.scalar.copy(k_sb, qk_ps[0:H])
                    nc.vector.tensor_copy(q_sb, qk_ps[H:P])

                    vtab = vtabs[pr % 2]
                    nc.vector.tensor_copy(vtab[0:H], vt_ps)

                    tr_ps = ps_tr.tile([P, 2, 2, HP1 + 1], bf16)
                    for s in range(2):
                        for j in range(2):
                            nc.tensor.transpose(
                                tr_ps[:, s, j, 0:HP1],
                                vtab[:, s, j * P : (j + 1) * P],
                                iden,
                            )
                    v_aug = vpool.tile([P, 2, 2, HP1], bf16)
                    nc.vector.tensor_copy(v_aug, tr_ps[:, :, :, 0:HP1])

                    p_sb = ppool.tile([P, 2, 3, P], bf16)
                    for s in range(2):
                        s_ps = ps_s.tile([P, 3, P], f32, name="s_ps")
                        # S^T[tk 0:128, tq 0:256]: stat = k^T chunk (K=64),
                        # moving = q^T from partitions 64:128
                        nc.tensor.matmul(
                            s_ps[:, 0:2, :],
                            k_sb[:, s, 0:P],
                            q_sb[:, s, :],
                            start=True,
                            stop=True,
                        )
                        # S^T[tk 128:256, tq 128:256]
                        nc.tensor.matmul(
                            s_ps[:, 2, :],
                            k_sb[:, s, P:T],
                            q_sb[:, s, P:T],
                            start=True,
                            stop=True,
                        )
                        nc.scalar.activation(
                            p_sb[:, s, :, :], s_ps, Exp, scale=0.125
                        )

                    # multiplicative causal mask, diagonal blocks only
                    nc.vector.tensor_tensor(
                        p_sb[:, :, 0, :],
                        p_sb[:, :, 0, :],
                        um[:, None, :].to_broadcast([P, 2, P]),
                        Mult,
                    )
                    nc.vector.tensor_tensor(
                        p_sb[:, :, 2, :],
                        p_sb[:, :, 2, :],
                        um[:, None, :].to_broadcast([P, 2, P]),
                        Mult,
                    )

                    o_ps = ps_o.tile([P, 2, 2, HP1], f32)
                    for s in range(2):
                        nc.tensor.matmul(
                            o_ps[:, s, 0, :],
                            p_sb[:, s, 0, :],
                            v_aug[:, s, 0, :],
                            start=True,
                            stop=True,
                        )
                        nc.tensor.matmul(
                            o_ps[:, s, 1, :],
                            p_sb[:, s, 1, :],
                            v_aug[:, s, 0, :],
                            start=True,
                            stop=False,
                        )
                        nc.tensor.matmul(
                            o_ps[:, s, 1, :],
                            p_sb[:, s, 2, :],
                            v_aug[:, s, 1, :],
                            start=False,
                            stop=True,
                        )

                    rec = rpool.tile([P, 2, 2, 1], f32)
                    nc.vector.reciprocal(rec, o_ps[:, :, :, H : H + 1])
                    nc.vector.tensor_tensor(
                        o_sb[:, s0 : s0 + 2, :, :],
                        o_ps[:, :, :, 0:H],
                        rec.to_broadcast([P, 2, 2, H]),
                        Mult,
                    )

                nc.sync.dma_start(out_d[:, qd], o_sb)

    nc.compile()
    return nc


def _prep_inputs(x, Wq, Wk, Wv, bpc):
    bf = ml_dtypes.bfloat16
    nb = NCORES * bpc
    nq = bpc // 4
    x = np.asarray(x, dtype=np.float32)[:nb]
    # [b, t, e] -> per core [qd, p, s, c, t] with b = qd*4+s, e = c*128+p
    xt = np.ascontiguousarray(
        x.reshape(NCORES, nq, 4, T, EC, P).transpose(0, 1, 5, 2, 4, 3)
    ).astype(bf)
    wkq = np.concatenate(
        [np.asarray(Wk, np.float32), np.asarray(Wq, np.float32)], axis=1
    )  # [E, 128]: k^T on PSUM partitions 0:64, q^T on 64:128
    wkq = np.ascontiguousarray(wkq.reshape(EC, P, P).transpose(1, 0, 2)).astype(bf)
    wv = np.ascontiguousarray(
        np.asarray(Wv, np.float32).reshape(EC, P, H).transpose(1, 0, 2)
    ).astype(bf)
    tril01 = (np.arange(P)[:, None] <= np.arange(P)[None, :]).astype(np.float32)
    um = tril01.astype(bf)
    iden = np.eye(HP1, dtype=np.float32).astype(bf)
    per_core = []
    for c in range(NCORES):
        per_core.append(
            {
                "xt": xt[c],
                "wkq": wkq,
                "wv": wv,
                "um": um,
                "iden": iden,
            }
        )
    return per_core


def kernel(x, Wq, Wk, Wv, _trace=False, _bpc=BPC):
    """Full inputs in, full output out. Shards batch dim over 8 NeuronCores."""
    from concourse import bass_utils

    if _trace:
        _install_ntff_hook()

    key = ("prog", _bpc)
    if key not in _cache:
        _cache[key] = _build_program(_bpc)
    nc = _cache[key]

    in_maps = _prep_inputs(x, Wq, Wk, Wv, _bpc)
    res = bass_utils.run_bass_kernel_spmd(
        nc, in_maps, core_ids=list(range(NCORES)), trace=_trace
    )
    _cache["last_result"] = res
    nq = _bpc // 4
    # device layout [p, qd, s, j, h] -> [b, t, h] with b=qd*4+s, t=j*128+p
    outs = []
    for r in res.results:
        o = r["out"].reshape(P, nq, 4, 2, H).transpose(1, 2, 3, 0, 4)
        outs.append(np.ascontiguousarray(o).reshape(_bpc, T, H))
    out = np.concatenate(outs, axis=0)
    return out.astype(np.float32)
